# revision 16
# baseline (speedup 1.0000x reference)
"""LIF spiking-neuron scan (SimpleSNN) Trainium2 Bass kernel.

Reference semantics (per sample b, neuron n, over T timesteps):
    mem = mem * 0.9 + x[t]
    spike[t] = (mem >= 1.5)
    mem = mem * (1 - spike[t])

Full inputs [256, 200, 1024] f32 are sharded batch-wise over 8 NeuronCores
(32 samples/core; the time recurrence is per-sample so no cross-core comms).

Host-side, each core's shard [32, 200, 1024] is permuted to a
partition-major layout [128, 200, 256] with partition p = k*32 + b
(k = n // 256, b = sample), so every chunk DMA is a single dense 3-D
transfer carrying one completion semaphore.

Per-core device design:
  - The recurrence is rewritten over the PRE-reset membrane w:
        w_t = select(w_{t-1} < 1.5, w_{t-1}, 0) * 0.9 + x_t
        spike_t = (w_t >= 1.5)
    which is bit-identical to the reference (same two f32 roundings per
    step) and needs only ONE fused custom-DVE op per step (5 ALU stages
    of the DVE's 8-stage pipeline). The w history is materialized in the
    chunk tile, so the whole sequential chain is 200 back-to-back Vector
    engine instructions at ~[128, 256] each.
  - T=200 steps split into chunks of 25. Per chunk: one DMA load of
    x [128, 25, 256], 25 fused LIF-step ops (DVE), one batched GpSimd
    tensor_scalar over the w chunk (spikes = (w >= 1.5) as 1.0/0.0),
    one DMA store of the spike chunk. All DMAs are SWDGE (gpsimd).
  - Bacc lowering splits multi-wait instructions into event-semaphore
    chains (TRN2 allows at most one sync wait per instruction).
"""

from contextlib import ExitStack

import numpy as np

B, T, N = 256, 200, 1024
NCORES = 8
BL = B // NCORES  # 32 samples per core
DECAY = 0.9
TH = 1.5
P128 = 128
FREE = 256  # free-dim size of the state tile
NK = N // FREE  # 4 n-blocks; partition p = k*32 + b
# Ragged chunking. The x loads (131 kB/step) at the ~360 GB/s per-core
# HBM cap run only ~20% faster than the serial DVE chain (~0.46
# us/step), so the loader builds slack slowly: start with a small chunk
# (chain starts ~3 us after the first bytes land) and grow
# geometrically. Small last chunk keeps the tail (final spike pass +
# store after the chain ends) short.
CHUNKS = [6, 8, 10, 12, 15, 18, 21, 25, 25, 25, 17, 10, 5, 3]
assert sum(CHUNKS) == T
TCMAX = max(CHUNKS)
# Spikes for all 200 steps stay resident in SBUF (T*FREE u8 = 50 KiB per
# partition) and are stored in a few large deferred DMAs, scheduled so
# the writes drain mostly after the loads finish: HBM read+write share
# the ~360 GB/s per-core budget, and interleaved per-chunk stores were
# measured to slow the loads below the chain rate (pipeline stalls).
# The shrinking tail chunks keep the post-chain work (last Sign + last
# store) tiny.
STORE_AFTER_CHUNK = {8: (0, 140), 10: (140, 182), 12: (182, 197), 13: (197, 200)}
for _c, (_a, _b) in STORE_AFTER_CHUNK.items():
    assert sum(CHUNKS[: _c + 1]) == _b

_CACHE = {}

_LIF_OP_NAME = "LIF_STEP_ANT"


def _lif_reference(in0, in1, s0, s1, imm2):
    return (
        np.where(in0 < np.float32(s0), in0, np.float32(0.0)) * np.float32(s1) + in1
    ).astype(np.float32)


def _register_lif_op():
    """Register the fused LIF-step custom DVE op:
        out = select(in0 < s0, in0, 0) * s1 + in1
    (in0 = previous membrane w, in1 = x_t, s0 = threshold, s1 = decay).
    Registration is the runtime equivalent of appending to dve_ops.OPS;
    uops_sha is computed from the same lower() used at compile time.
    """
    import concourse.dve_ops as dve_ops
    from concourse.dve_ops import DveOp
    from concourse.dve_spec import C0, C1, Spec, Src0, Src1, Zero, lower, select
    from concourse.dve_uop import DveOpSpec

    if _LIF_OP_NAME in dve_ops._SUB_OPCODE_FOR_NAME:
        for op in dve_ops.OPS:
            if op.name == _LIF_OP_NAME:
                return op
        raise RuntimeError("LIF op registered but not in OPS")

    body = select(Src0 < C0, Src0, Zero) * C1 + Src1
    spec = Spec(body=body, reference=_lif_reference)
    row = dve_ops._CUSTOM_DVE_ROW_BASE + len(dve_ops.OPS)
    shas = {}
    for ver in ("v3", "v4"):
        uops = lower(spec, ver=ver)
        shas[ver] = DveOpSpec(
            name=_LIF_OP_NAME, opcode=row, uops=uops, rd1_en=True
        ).sha(ver)
    op = DveOp(_LIF_OP_NAME, spec, subdim=False, uops_sha=shas)
    dve_ops.OPS.append(op)
    dve_ops._SUB_OPCODE_FOR_NAME[_LIF_OP_NAME] = row
    dve_ops.CUSTOM_DVE_SPECS[_LIF_OP_NAME] = spec
    return op


def _build_bass(reps: int = 1):
    # reps > 1 repeats the whole pipeline on the same buffers (benchmarking
    # only — amortizes host dispatch overhead to expose the device time).
    import concourse.bacc as bacc
    import concourse.tile as tile
    from concourse import mybir

    lif_op = _register_lif_op()

    nc = bacc.Bacc(
        "TRN2",
        target_bir_lowering=False,
        debug=False,
        enable_asserts=False,
    )

    P = P128
    f32 = mybir.dt.float32

    u8 = mybir.dt.uint8
    x_d = nc.dram_tensor("x", [P, T, FREE], f32, kind="ExternalInput").ap()
    s_d = nc.dram_tensor("spk", [P, T, FREE], u8, kind="ExternalOutput").ap()

    with ExitStack() as ctx:
        tc = ctx.enter_context(tile.TileContext(nc))
        xp = ctx.enter_context(tc.tile_pool(name="xp", bufs=2))
        wp = ctx.enter_context(tc.tile_pool(name="wp", bufs=3))
        sp = ctx.enter_context(tc.tile_pool(name="sp", bufs=1))
        st = ctx.enter_context(tc.tile_pool(name="st", bufs=1))

        zero = st.tile([P, FREE], f32)
        nc.vector.memset(zero[:], 0.0)
        # Spike threshold as an ACT bias: sign(w + SPIKE_BIAS) is +1 exactly
        # when w >= TH (SPIKE_BIAS = nextafter(-TH, 0), so w == TH lands one
        # ulp above zero and w == TH - 1ulp lands exactly on zero -> sign 0).
        spike_bias = st.tile([P, 1], f32, tag="bias")
        nc.vector.memset(
            spike_bias[:], float(np.nextafter(np.float32(-TH), np.float32(0)))
        )

        # All spikes accumulate here; stored by a few large deferred DMAs.
        spk_all = sp.tile([P, T, FREE], u8)

        wt_prev = None
        prev_tc = None
        for c, tcsz in enumerate(CHUNKS * reps):
            t0 = sum(CHUNKS[: c % len(CHUNKS)])
            xt = xp.tile([P, TCMAX, FREE], f32, tag="x")
            # Loads ride the SP HWDGE ring, stores the ACT HWDGE ring —
            # two independent DMA queues that overlap.
            nc.sync.dma_start(out=xt[:, :tcsz, :], in_=x_d[:, t0 : t0 + tcsz, :])

            wt = wp.tile([P, TCMAX, FREE], f32, tag="w")
            for j in range(tcsz):
                if c == 0 and j == 0:
                    w_in = zero[:]
                elif j == 0:
                    w_in = wt_prev[:, prev_tc - 1, :]
                else:
                    w_in = wt[:, j - 1, :]
                # w_t = select(w_{t-1} < TH, w_{t-1}, 0) * DECAY + x_t
                nc.vector._custom_dve(
                    lif_op,
                    out=wt[:, j, :],
                    in0=w_in,
                    in1=xt[:, j, :],
                    s0=TH,
                    s1=DECAY,
                )
            wt_prev = wt
            prev_tc = tcsz

            # spikes as sign(w + SPIKE_BIAS) in {-1, 0, +1} stored u8 (the
            # host maps ==1 -> 1.0f). Runs on the otherwise-idle Scalar
            # engine; GpSimd's tensor_scalar measures ~18 cyc/elem and
            # serializes the kernel, ACT streams at 1 elem/cycle.
            nc.scalar.activation(
                out=spk_all[:, t0 : t0 + tcsz, :].rearrange("p t f -> p (t f)"),
                in_=wt[:, :tcsz, :].rearrange("p t f -> p (t f)"),
                func=mybir.ActivationFunctionType.Sign,
                bias=spike_bias[:],
            )
            if c % len(CHUNKS) in STORE_AFTER_CHUNK:
                a, b = STORE_AFTER_CHUNK[c % len(CHUNKS)]
                nc.scalar.dma_start(
                    out=s_d[:, a:b, :], in_=spk_all[:, a:b, :]
                )

    # Bacc lowering: splits multi-wait instructions into event-semaphore
    # chains (TRN2 allows at most one sync wait per instruction), register
    # allocation, DCE.
    nc.compile()
    return nc


def _get_nc():
    if "nc" not in _CACHE:
        _CACHE["nc"] = _build_bass()
    return _CACHE["nc"]


def _shard_input(inputs: np.ndarray, i: int) -> np.ndarray:
    # [32, 200, 1024] -> [32, 200, 4, 256] -> [4, 32, 200, 256] -> [128, 200, 256]
    xi = inputs[i * BL : (i + 1) * BL]
    xi = xi.reshape(BL, T, NK, FREE).transpose(2, 0, 1, 3)
    return np.ascontiguousarray(xi).reshape(P128, T, FREE)


def _unshard_output(spk: np.ndarray) -> np.ndarray:
    # [128, 200, 256] u8 -> [4, 32, 200, 256] -> [32, 200, 4, 256] -> [32, 200, 1024]
    s = spk.reshape(NK, BL, T, FREE).transpose(1, 2, 0, 3)
    return np.ascontiguousarray(s).reshape(BL, T, N)


def kernel(inputs: np.ndarray, trace: bool = False) -> np.ndarray:
    from concourse.bass_utils import run_bass_kernel_spmd

    inputs = np.ascontiguousarray(np.asarray(inputs, dtype=np.float32))
    assert inputs.shape == (B, T, N), inputs.shape

    nc = _get_nc()
    in_maps = [{"x": _shard_input(inputs, i)} for i in range(NCORES)]
    res = run_bass_kernel_spmd(
        nc, in_maps, core_ids=list(range(NCORES)), trace=trace
    )
    _CACHE["last_results"] = res
    out = np.concatenate(
        [_unshard_output(r["spk"]) for r in res.results], axis=0
    )
    # Device stores sign(w + SPIKE_BIAS) as u8: +1 (= spike) maps to 1,
    # 0 and -1 (however the f32->u8 conversion encodes it) map to not-1.
    return (out == 1).astype(np.float32)



# revision 21
# speedup vs baseline: 1.1942x; 1.1942x over previous
"""LIF spiking-neuron scan (SimpleSNN) Trainium2 Bass kernel.

Reference semantics (per sample b, neuron n, over T timesteps):
    mem = mem * 0.9 + x[t]
    spike[t] = (mem >= 1.5)
    mem = mem * (1 - spike[t])

Full inputs [256, 200, 1024] f32 are sharded batch-wise over 8 NeuronCores
(32 samples/core; the time recurrence is per-sample so no cross-core comms).

Host-side, each core's shard [32, 200, 1024] is permuted to a
partition-major layout [128, 200, 256] with partition p = k*32 + b
(k = n // 256, b = sample), so every chunk DMA is a single dense 3-D
transfer carrying one completion semaphore.

Per-core device design:
  - The recurrence is rewritten over the PRE-reset membrane w:
        w_t = select(w_{t-1} < 1.5, w_{t-1}, 0) * 0.9 + x_t
        spike_t = (w_t >= 1.5)
    which is bit-identical to the reference (same two f32 roundings per
    step) and needs only ONE fused custom-DVE op per step (5 ALU stages
    of the DVE's 8-stage pipeline). The w history is materialized in the
    chunk tile, so the whole sequential chain is 200 back-to-back Vector
    engine instructions at ~[128, 256] each.
  - T=200 steps split into chunks of 25. Per chunk: one DMA load of
    x [128, 25, 256], 25 fused LIF-step ops (DVE), one batched GpSimd
    tensor_scalar over the w chunk (spikes = (w >= 1.5) as 1.0/0.0),
    one DMA store of the spike chunk. All DMAs are SWDGE (gpsimd).
  - Bacc lowering splits multi-wait instructions into event-semaphore
    chains (TRN2 allows at most one sync wait per instruction).
"""

from contextlib import ExitStack

import numpy as np

B, T, N = 256, 200, 1024
NCORES = 8
BL = B // NCORES  # 32 samples per core
DECAY = 0.9
TH = 1.5
P128 = 128
FREE = 256  # free-dim size of the state tile
NK = N // FREE  # 4 n-blocks; partition p = k*32 + b
# Ragged chunking. The x loads (131 kB/step) at the ~360 GB/s per-core
# HBM cap run only ~20% faster than the serial DVE chain (~0.46
# us/step), so the loader builds slack slowly: start with a small chunk
# (chain starts ~3 us after the first bytes land) and grow
# geometrically. Small last chunk keeps the tail (final spike pass +
# store after the chain ends) short.
CHUNKS = [6, 8, 10, 12, 15, 18, 21, 25, 25, 25, 17, 10, 5, 3]
assert sum(CHUNKS) == T
TCMAX = max(CHUNKS)
# Spikes for all 200 steps stay resident in SBUF (T*FREE u8 = 50 KiB per
# partition) and are stored in a few large deferred DMAs, scheduled so
# the writes drain mostly after the loads finish: HBM read+write share
# the ~360 GB/s per-core budget, and interleaved per-chunk stores were
# measured to slow the loads below the chain rate (pipeline stalls).
# Each store group gets its OWN SBUF tile: a single shared tile made
# Tile's whole-tile WAR tracking stall later Sign ops behind earlier
# groups' store reads (measured 6.8 us chain stall). The shrinking tail
# chunks keep the post-chain work (last Sign + last store) tiny.
STORE_AFTER_CHUNK = {8: (0, 140), 10: (140, 182), 12: (182, 197), 13: (197, 200)}
for _c, (_a, _b) in STORE_AFTER_CHUNK.items():
    assert sum(CHUNKS[: _c + 1]) == _b

_CACHE = {}

_LIF_OP_NAME = "LIF_STEP_ANT"


def _lif_reference(in0, in1, s0, s1, imm2):
    return (
        np.where(in0 < np.float32(s0), in0, np.float32(0.0)) * np.float32(s1) + in1
    ).astype(np.float32)


def _register_lif_op():
    """Register the fused LIF-step custom DVE op:
        out = select(in0 < s0, in0, 0) * s1 + in1
    (in0 = previous membrane w, in1 = x_t, s0 = threshold, s1 = decay).
    Registration is the runtime equivalent of appending to dve_ops.OPS;
    uops_sha is computed from the same lower() used at compile time.
    """
    import concourse.dve_ops as dve_ops
    from concourse.dve_ops import DveOp
    from concourse.dve_spec import C0, C1, Spec, Src0, Src1, Zero, lower, select
    from concourse.dve_uop import DveOpSpec

    if _LIF_OP_NAME in dve_ops._SUB_OPCODE_FOR_NAME:
        for op in dve_ops.OPS:
            if op.name == _LIF_OP_NAME:
                return op
        raise RuntimeError("LIF op registered but not in OPS")

    body = select(Src0 < C0, Src0, Zero) * C1 + Src1
    spec = Spec(body=body, reference=_lif_reference)
    row = dve_ops._CUSTOM_DVE_ROW_BASE + len(dve_ops.OPS)
    shas = {}
    for ver in ("v3", "v4"):
        uops = lower(spec, ver=ver)
        shas[ver] = DveOpSpec(
            name=_LIF_OP_NAME, opcode=row, uops=uops, rd1_en=True
        ).sha(ver)
    op = DveOp(_LIF_OP_NAME, spec, subdim=False, uops_sha=shas)
    dve_ops.OPS.append(op)
    dve_ops._SUB_OPCODE_FOR_NAME[_LIF_OP_NAME] = row
    dve_ops.CUSTOM_DVE_SPECS[_LIF_OP_NAME] = spec
    return op


def _build_bass(reps: int = 1):
    # reps > 1 repeats the whole pipeline on the same buffers (benchmarking
    # only — amortizes host dispatch overhead to expose the device time).
    import concourse.bacc as bacc
    import concourse.tile as tile
    from concourse import mybir

    lif_op = _register_lif_op()

    nc = bacc.Bacc(
        "TRN2",
        target_bir_lowering=False,
        debug=False,
        enable_asserts=False,
    )

    P = P128
    f32 = mybir.dt.float32

    u8 = mybir.dt.uint8
    x_d = nc.dram_tensor("x", [P, T, FREE], f32, kind="ExternalInput").ap()
    s_d = nc.dram_tensor("spk", [P, T, FREE], u8, kind="ExternalOutput").ap()

    with ExitStack() as ctx:
        tc = ctx.enter_context(tile.TileContext(nc))
        xp = ctx.enter_context(tc.tile_pool(name="xp", bufs=3))
        wp = ctx.enter_context(tc.tile_pool(name="wp", bufs=2))
        sp = ctx.enter_context(tc.tile_pool(name="sp", bufs=1))
        st = ctx.enter_context(tc.tile_pool(name="st", bufs=1))

        zero = st.tile([P, FREE], f32)
        nc.vector.memset(zero[:], 0.0)
        # Spike threshold as an ACT bias: sign(w + SPIKE_BIAS) is +1 exactly
        # when w >= TH (SPIKE_BIAS = nextafter(-TH, 0), so w == TH lands one
        # ulp above zero and w == TH - 1ulp lands exactly on zero -> sign 0).
        spike_bias = st.tile([P, 1], f32, tag="bias")
        nc.vector.memset(
            spike_bias[:], float(np.nextafter(np.float32(-TH), np.float32(0)))
        )

        # Spikes accumulate in one tile per store group (deferred DMAs).
        spk_group = {}  # chunk index whose completion triggers the store -> (tile, a, b)
        for _c, (a, b) in STORE_AFTER_CHUNK.items():
            spk_group[_c] = (
                sp.tile([P, b - a, FREE], u8, name=f"spk{_c}", tag=f"s{_c}"),
                a,
                b,
            )

        def group_of(t):
            for _c, (tile_, a, b) in spk_group.items():
                if a <= t < b:
                    return tile_, a, b
            raise AssertionError(t)

        wt_prev = None
        prev_tc = None
        for c, tcsz in enumerate(CHUNKS * reps):
            t0 = sum(CHUNKS[: c % len(CHUNKS)])
            xt = xp.tile([P, TCMAX, FREE], f32, tag="x")
            # Loads ride the SP HWDGE ring, stores the ACT HWDGE ring —
            # two independent DMA queues that overlap.
            nc.sync.dma_start(out=xt[:, :tcsz, :], in_=x_d[:, t0 : t0 + tcsz, :])

            wt = wp.tile([P, TCMAX, FREE], f32, tag="w")
            for j in range(tcsz):
                if c == 0 and j == 0:
                    w_in = zero[:]
                elif j == 0:
                    w_in = wt_prev[:, prev_tc - 1, :]
                else:
                    w_in = wt[:, j - 1, :]
                # w_t = select(w_{t-1} < TH, w_{t-1}, 0) * DECAY + x_t
                nc.vector._custom_dve(
                    lif_op,
                    out=wt[:, j, :],
                    in0=w_in,
                    in1=xt[:, j, :],
                    s0=TH,
                    s1=DECAY,
                )
            wt_prev = wt
            prev_tc = tcsz

            # spikes as sign(w + SPIKE_BIAS) in {-1, 0, +1} stored u8 (the
            # host maps ==1 -> 1.0f). Runs on the otherwise-idle Scalar
            # engine; GpSimd's tensor_scalar measures ~18 cyc/elem and
            # serializes the kernel, ACT streams at 1 elem/cycle.
            gt, ga, gb = group_of(t0)
            assert t0 + tcsz <= gb, "chunk spans store groups"
            nc.scalar.activation(
                out=gt[:, t0 - ga : t0 - ga + tcsz, :].rearrange(
                    "p t f -> p (t f)"
                ),
                in_=wt[:, :tcsz, :].rearrange("p t f -> p (t f)"),
                func=mybir.ActivationFunctionType.Sign,
                bias=spike_bias[:],
            )
            if c % len(CHUNKS) in STORE_AFTER_CHUNK:
                a, b = STORE_AFTER_CHUNK[c % len(CHUNKS)]
                nc.scalar.dma_start(out=s_d[:, a:b, :], in_=gt[:])

    # Bacc lowering: splits multi-wait instructions into event-semaphore
    # chains (TRN2 allows at most one sync wait per instruction), register
    # allocation, DCE.
    nc.compile()
    return nc


def _get_nc():
    if "nc" not in _CACHE:
        _CACHE["nc"] = _build_bass()
    return _CACHE["nc"]


def _shard_input(inputs: np.ndarray, i: int) -> np.ndarray:
    # [32, 200, 1024] -> [32, 200, 4, 256] -> [4, 32, 200, 256] -> [128, 200, 256]
    xi = inputs[i * BL : (i + 1) * BL]
    xi = xi.reshape(BL, T, NK, FREE).transpose(2, 0, 1, 3)
    return np.ascontiguousarray(xi).reshape(P128, T, FREE)


def _unshard_output(spk: np.ndarray) -> np.ndarray:
    # [128, 200, 256] u8 -> [4, 32, 200, 256] -> [32, 200, 4, 256] -> [32, 200, 1024]
    s = spk.reshape(NK, BL, T, FREE).transpose(1, 2, 0, 3)
    return np.ascontiguousarray(s).reshape(BL, T, N)


def kernel(inputs: np.ndarray, trace: bool = False) -> np.ndarray:
    from concourse.bass_utils import run_bass_kernel_spmd

    inputs = np.ascontiguousarray(np.asarray(inputs, dtype=np.float32))
    assert inputs.shape == (B, T, N), inputs.shape

    nc = _get_nc()
    in_maps = [{"x": _shard_input(inputs, i)} for i in range(NCORES)]
    res = run_bass_kernel_spmd(
        nc, in_maps, core_ids=list(range(NCORES)), trace=trace
    )
    _CACHE["last_results"] = res
    out = np.concatenate(
        [_unshard_output(r["spk"]) for r in res.results], axis=0
    )
    # Device stores sign(w + SPIKE_BIAS) as u8: +1 (= spike) maps to 1,
    # 0 and -1 (however the f32->u8 conversion encodes it) map to not-1.
    return (out == 1).astype(np.float32)

